# revision 5
# baseline (speedup 1.0000x reference)
"""Trainium2 Bass kernel for nn_BSLoss (text-snake style OHEM loss), 8-core
data-parallel.

Strategy
--------
Host shards the batch dim (16 -> 2 per core). gtm masks ship fp8-e3m4 (0/1
exact), cls logits bf16, and the 32 regression channels (map + pred) ship
fp8-e3m4 channel-innermost per pixel, with map and pred concatenated per
chunk ([128, 64*Fc] = [gtx | reg]) so each chunk is ONE fat-row DMA.

Device per core:
  - DVE: masks + counts (in the pre-data idle window), then one fused custom
    op per chunk computing q = 2*smooth_l1(gtx-reg) AND a continuous running
    sum (scan) in a single 1x pass; chunks chain their cumsum via the scan's
    s0 init read from the previous chunk's last column, so each level has one
    continuous cumsum C. Per-level weighted totals come Abel-style from three
    small strided reductions: A = sum_f w_f C[32f+15], B = sum_{f>=1} w_f
    C[32f-1], D = sum_f w_f C[32f+31]; host computes x = A-B, y = D-A.
  - GPSIMD: logit diffs, d_ce, pos*ce products, vn = (ce+1)*neg.
  - ScalarE: sgn, exp, ln(1+x) (2-class CE), and the pos*ce accumulations.
Host merges partials, does the exact global top-k OHEM over the masked
negative CE values (vn, bf16) and the final divisions.
"""

import numpy as np
import ml_dtypes

import concourse.bacc as bacc
import concourse.mybir as mybir
import concourse.dve_ops as dve_ops
from concourse.dve_spec import (
    Spec, Src0, Src1, C0, Zero, One, AluOp, Bin, minn, scan, lower, _has_src1,
)
from concourse.dve_uop import DveOpSpec
from concourse import tile

F32 = mybir.dt.float32
BF16 = mybir.dt.bfloat16
FP8 = mybir.dt.float8e3
NP_BF16 = ml_dtypes.bfloat16
NP_FP8 = ml_dtypes.float8_e3m4
ALU = mybir.AluOpType
ACT = mybir.ActivationFunctionType

NCORES = 8
B_PER_CORE = 2
# level -> (H, W, nchunks)
LEVELS = [(3, 160, 160, 4), (4, 80, 80, 1), (5, 40, 40, 1)]
FS = [B_PER_CORE * H * W // 128 for _, H, W, _ in LEVELS]   # [400, 100, 25]
KCH = 16          # regression channels per axis
OHEM_RATIO = 3.0

# stats tile column layout: per-level block of 5, then 3 cols per level
# (A, B, D weighted-cumsum reductions). Every column is written exactly once.
C_NPOS, C_NEGCNT, C_LOSSPOS, C_TCLPOS, C_TCLALL = range(5)
N_LEVEL_COLS = 5 * len(LEVELS)
STATS_COLS = N_LEVEL_COLS + 3 * len(LEVELS)


def _np_sl1q(d):
    a = np.abs(d)
    m = np.minimum(a, 1.0)
    return m * (a + a - m)   # == 2 * smooth_l1(d)


def _register_custom_ops():
    """Register our fused DVE ops (idempotent)."""
    # QSL1CS: out = s0 + cumsum_freedim(q(Src0 - Src1)),
    #         q(d) = min(|d|,1)*(2|d| - min(|d|,1))
    a = Bin(AluOp.ABSOLUTE_DIFF, Src0, Src1)
    m = minn(a, One)
    q = ((a + a) - m) * m

    def _qs_ref(in0, in1, s0, s1, imm2):
        p = in0.shape[0]
        qq = _np_sl1q(in0.reshape(p, -1).astype(np.float32)
                      - in1.reshape(p, -1).astype(np.float32))
        init = np.asarray(s0).reshape(-1, 1) if isinstance(s0, np.ndarray) else s0
        return init + np.cumsum(qq, axis=1)

    spec_qs = Spec(body=scan(AluOp.ADD, q, init=C0), reference=_qs_ref)

    def _acc_ref(fn):
        def ref(in0, in1, s0, s1, imm2):
            p = in0.shape[0]
            o = fn(in0.reshape(p, -1).astype(np.float32),
                   in1.reshape(p, -1).astype(np.float32) if in1 is not None
                   else None)
            init = np.asarray(s0).reshape(-1, 1) if isinstance(s0, np.ndarray) else s0
            return o, init + o.sum(axis=1, keepdims=True)
        return ref

    # MULR: out = in0*in1 ; accum = s0 + sum(out)
    spec_mulr = Spec(body=Src0 * Src1, accum=AluOp.ADD, accum_init=C0,
                     reference=_acc_ref(lambda a_, b_: a_ * b_))
    # NEGM: out = (1-in0)*in1 ; accum = s0 + sum(out)
    spec_negm = Spec(body=(One - Src0) * Src1, accum=AluOp.ADD, accum_init=C0,
                     reference=_acc_ref(lambda a_, b_: (1.0 - a_) * b_))

    ops = {}
    for name, spec in (("QSL1CS_ANT", spec_qs), ("MULR_ANT", spec_mulr),
                       ("NEGM_ANT", spec_negm)):
        if name in dve_ops._SUB_OPCODE_FOR_NAME:
            ops[name] = next(o for o in dve_ops.OPS if o.name == name)
            continue
        row = dve_ops._CUSTOM_DVE_ROW_BASE + len(dve_ops.OPS)
        shas = {}
        for ver in ("v3", "v4"):
            u = lower(spec, ver=ver)
            shas[ver] = DveOpSpec(name=name, opcode=row, uops=u,
                                  rd1_en=_has_src1(spec)).sha(ver)
        op = dve_ops.DveOp(name, spec, subdim=False, uops_sha=shas)
        dve_ops.OPS.append(op)
        dve_ops.CUSTOM_DVE_SPECS[name] = spec
        dve_ops._SUB_OPCODE_FOR_NAME[name] = row
        ops[name] = op
    return ops


def _install_act_root():
    """Restrict the ACT table universe to the one set holding every function
    we use (exp, ln, identity, copy), so walrus never ping-pongs table sets."""
    import os, json, shutil, tempfile
    if os.environ.get("BASS_ACT_ROOT_JSON_PATH"):
        return
    try:
        from neuronxcc.driver.Job import Job
        from neuronxcc.driver.jobs.support.FindActInfo import findActInfoFile
        src = findActInfoFile(Job.getPackageDir(), "gen3")
        d = json.load(open(src))
        keep = [t for t in d["act_func_sets"]
                if t["name"] == "natural_log_exp_and_others"]
        if not keep:
            return
        tmp = tempfile.mkdtemp(prefix="act_root_")
        srcdir = os.path.dirname(src)
        for t in keep:
            for k in d["pwp_file_keys"]:
                shutil.copy(os.path.join(srcdir, t[k]), tmp)
        with open(os.path.join(tmp, "act_info.json"), "w") as f:
            json.dump({"pwp_file_keys": d["pwp_file_keys"],
                       "act_func_sets": keep}, f)
        os.environ["BASS_ACT_ROOT_JSON_PATH"] = os.path.join(tmp, "act_info.json")
        # Make bass's pre-placed LoadActFuncSet ids consistent with the
        # stripped act_info: patch the table universe to the single set.
        import concourse.hw_specs as hw_specs
        _orig_gat = hw_specs.get_activation_tables

        def _gat(module_arch):
            full = _orig_gat(module_arch)
            return {"natural_log_exp_and_others":
                    full["natural_log_exp_and_others"]}

        hw_specs.get_activation_tables = _gat
        import concourse.bacc as _bacc_mod
        _bacc_mod.get_activation_tables = _gat
        import concourse.bass_interp as _bi_mod
        _bi_mod.get_activation_tables = _gat
    except Exception:
        pass


def build_bass():
    """Build the SPMD Bass module (one core's program)."""
    _install_act_root()
    ops = _register_custom_ops()
    nc = bacc.Bacc("TRN2")

    GTM_COLS = 3 * sum(FS)            # 1575
    CLS_COLS = 4 * sum(FS)            # 2100

    dram_in = {}
    dram_out = {}
    dram_in["gtma"] = nc.dram_tensor("gtma", [128, GTM_COLS], FP8,
                                     kind="ExternalInput")
    dram_in["clsa"] = nc.dram_tensor("clsa", [128, CLS_COLS], BF16,
                                     kind="ExternalInput")
    for lvl, H, W, nch in LEVELS:
        F = B_PER_CORE * H * W // 128
        Fc = F // nch
        for j in range(nch):
            dram_in[f"ch{lvl}_{j}"] = nc.dram_tensor(
                f"ch{lvl}_{j}", [128, 64 * Fc], FP8, kind="ExternalInput")
        dram_out[f"vn{lvl}"] = nc.dram_tensor(
            f"vn{lvl}", [128, F], BF16, kind="ExternalOutput")
    dram_out["stats"] = nc.dram_tensor(
        "stats", [128, STATS_COLS], F32, kind="ExternalOutput")

    QSL1CS, MULR, NEGM = ops["QSL1CS_ANT"], ops["MULR_ANT"], ops["NEGM_ANT"]

    with tile.TileContext(nc) as tc:
        with (
            tc.tile_pool(name="io", bufs=1) as io,
            tc.tile_pool(name="lv", bufs=1) as lv,
            tc.tile_pool(name="wk", bufs=1) as wk,
            tc.tile_pool(name="st", bufs=1) as stp,
        ):
            stats = stp.tile([128, STATS_COLS], F32, name="stats_t")

            # ---- loads: gtm first, then chunks (L3..L5), cls last ----
            GTMA = lv.tile([128, GTM_COLS], FP8, tag="gtma", name="gtma_t")
            CLSA = lv.tile([128, CLS_COLS], BF16, tag="clsa", name="clsa_t")
            CH = {}
            for lvl, H, W, nch in LEVELS:
                F = B_PER_CORE * H * W // 128
                Fc = F // nch
                for j in range(nch):
                    CH[(lvl, j)] = io.tile([128, 64 * Fc], FP8,
                                           tag=f"ch{lvl}_{j}",
                                           name=f"ch_{lvl}_{j}")
            nc.sync.dma_start(GTMA[:, :], dram_in["gtma"][:, :])
            for lvl, H, W, nch in LEVELS:
                for j in range(nch):
                    nc.sync.dma_start(CH[(lvl, j)][:, :],
                                      dram_in[f"ch{lvl}_{j}"][:, :])
            nc.sync.dma_start(CLSA[:, :], dram_in["clsa"][:, :])

            # per-level views into gtma/clsa
            goff = [3 * sum(FS[:i]) for i in range(len(LEVELS))]
            coff = [4 * sum(FS[:i]) for i in range(len(LEVELS))]

            # ---- level tiles ----
            POS, NEG, W2, SGN, DIFF, DCE, EXPD, CE, CEP, VN, QT = (
                {}, {}, {}, {}, {}, {}, {}, {}, {}, {}, {})
            for li, (lvl, H, W, nch) in enumerate(LEVELS):
                F = FS[li]
                POS[li] = lv.tile([128, F], F32, tag=f"pos{li}", name=f"pos_{lvl}")
                NEG[li] = lv.tile([128, F], F32, tag=f"neg{li}", name=f"neg_{lvl}")
                W2[li] = lv.tile([128, F], F32, tag=f"w2{li}", name=f"w2_{lvl}")
                SGN[li] = lv.tile([128, 2 * F], BF16, tag=f"sgn{li}",
                                  name=f"sgn_{lvl}")
                DIFF[li] = lv.tile([128, 2 * F], BF16, tag=f"diff{li}",
                                   name=f"diff_{lvl}")
                DCE[li] = lv.tile([128, 2 * F], BF16, tag=f"dce{li}",
                                  name=f"dce_{lvl}")
                EXPD[li] = lv.tile([128, 2 * F], F32, tag=f"expd{li}",
                                   name=f"expd_{lvl}")
                CE[li] = lv.tile([128, 2 * F], F32, tag=f"ce{li}",
                                 name=f"ce_{lvl}")
                CEP[li] = lv.tile([128, 2 * F], F32, tag=f"cep{li}",
                                  name=f"cep_{lvl}")
                VN[li] = lv.tile([128, F], BF16, tag=f"vn{li}", name=f"vn_{lvl}")
                QT[li] = wk.tile([128, 32 * F], F32, tag=f"q{li}",
                                 name=f"q_{lvl}")

            junk = lv.tile([128, 2 * max(FS)], BF16, tag="junk", name="junk_t")
            vnt = lv.tile([128, max(FS)], BF16, tag="vnt", name="vnt_t")

            # ---- DVE: masks + counts for all levels (pre-data window) ----
            for li, (lvl, H, W, nch) in enumerate(LEVELS):
                F = FS[li]
                g0 = goff[li]
                tr = GTMA[:, g0:g0 + F]
                train = GTMA[:, g0 + 2 * F:g0 + 3 * F]
                tcl = GTMA[:, g0 + F:g0 + 2 * F]
                base = 5 * li
                nc.vector._custom_dve(
                    MULR, out=POS[li][:, :], in0=tr, in1=train, s0=0.0,
                    accum_out=stats[:, base + C_NPOS:base + C_NPOS + 1])
                nc.vector._custom_dve(
                    NEGM, out=NEG[li][:, :], in0=tr, in1=train, s0=0.0,
                    accum_out=stats[:, base + C_NEGCNT:base + C_NEGCNT + 1])
                # w2 = (1 + tcl) * pos  ==  (tr + tcl) * pos  for 0/1 masks
                nc.vector.scalar_tensor_tensor(
                    out=W2[li][:, :], in0=tcl, scalar=1.0, in1=POS[li][:, :],
                    op0=ALU.add, op1=ALU.mult)

            # ---- ScalarE: sgn for all levels (needs gtm only) ----
            for li, (lvl, H, W, nch) in enumerate(LEVELS):
                F = FS[li]
                g0 = goff[li]
                nc.scalar.activation(SGN[li][:, :], GTMA[:, g0:g0 + 2 * F],
                                     ACT.Identity, bias=1.0, scale=-2.0)

            # ---- GPSIMD diff/dce + ScalarE exp/ln per level ----
            for li, (lvl, H, W, nch) in enumerate(LEVELS):
                F = FS[li]
                c0 = coff[li]
                base = 5 * li
                cls3d = CLSA[:, c0:c0 + 4 * F].rearrange(
                    "p (g f) -> p g f", g=2)
                nc.gpsimd.tensor_tensor(
                    out=DIFF[li][:, :].rearrange("p (g f) -> p g f", g=2),
                    in0=cls3d[:, :, F:2 * F], in1=cls3d[:, :, 0:F],
                    op=ALU.subtract)
                nc.gpsimd.tensor_mul(DCE[li][:, :], DIFF[li][:, :],
                                     SGN[li][:, :])
                nc.scalar.activation(EXPD[li][:, :], DCE[li][:, :], ACT.Exp)
                nc.scalar.activation(CE[li][:, 0:F], EXPD[li][:, 0:F],
                                     ACT.Ln, bias=1.0)
                nc.scalar.activation(
                    CE[li][:, F:2 * F], EXPD[li][:, F:2 * F], ACT.Ln,
                    bias=1.0,
                    accum_out=stats[:, base + C_TCLALL:base + C_TCLALL + 1])

            # ---- DVE: per-chunk fused q+cumsum (chained) + per-level trios
            # ---- GPSIMD: cep/vn; ScalarE: cep accums; stores per level
            for li, (lvl, H, W, nch) in enumerate(LEVELS):
                F = FS[li]
                Fc = F // nch
                base = 5 * li
                rb = N_LEVEL_COLS + 3 * li
                Q = QT[li]
                for j in range(nch):
                    s0 = (0.0 if j == 0
                          else Q[:, j * 32 * Fc - 1:j * 32 * Fc])
                    nc.vector._custom_dve(
                        QSL1CS, out=Q[:, j * 32 * Fc:(j + 1) * 32 * Fc],
                        in0=CH[(lvl, j)][:, 0:32 * Fc],
                        in1=CH[(lvl, j)][:, 32 * Fc:64 * Fc], s0=s0)
                QL = Q[:, :].rearrange("p (s c) -> p s c", c=32)
                w2a = W2[li][:, :].unsqueeze(2)
                scr = wk.tile([128, F], F32, tag=f"scr{li}", name=f"scr_{lvl}")
                # A = sum_f w_f C[32f+15]
                nc.vector._custom_dve(
                    MULR, out=scr[:, :].unsqueeze(2), in0=QL[:, :, 15:16],
                    in1=w2a, s0=0.0, accum_out=stats[:, rb:rb + 1])
                # B = sum_{f>=1} w_f C[32(f-1)+31]
                nc.vector._custom_dve(
                    MULR, out=scr[:, 0:F - 1].unsqueeze(2),
                    in0=QL[:, 0:F - 1, 31:32],
                    in1=W2[li][:, 1:F].unsqueeze(2), s0=0.0,
                    accum_out=stats[:, rb + 1:rb + 2])
                # D = sum_f w_f C[32f+31]
                nc.vector._custom_dve(
                    MULR, out=scr[:, :].unsqueeze(2), in0=QL[:, :, 31:32],
                    in1=w2a, s0=0.0, accum_out=stats[:, rb + 2:rb + 3])

                # cep = pos * ce (both halves); accumulate on ScalarE
                nc.gpsimd.tensor_mul(CEP[li][:, 0:F], POS[li][:, :],
                                     CE[li][:, 0:F])
                nc.gpsimd.tensor_mul(CEP[li][:, F:2 * F], POS[li][:, :],
                                     CE[li][:, F:2 * F])
                nc.scalar.activation(
                    junk[:, 0:F], CEP[li][:, 0:F], ACT.Identity,
                    accum_out=stats[:, base + C_LOSSPOS:base + C_LOSSPOS + 1])
                nc.scalar.activation(
                    junk[:, F:2 * F], CEP[li][:, F:2 * F], ACT.Identity,
                    accum_out=stats[:, base + C_TCLPOS:base + C_TCLPOS + 1])
                # vn = (ce_tr + 1) * neg == ce_tr*neg + neg
                nc.gpsimd.tensor_mul(vnt[:, 0:F], CE[li][:, 0:F],
                                     NEG[li][:, :])
                nc.gpsimd.tensor_tensor(out=VN[li][:, :], in0=vnt[:, 0:F],
                                        in1=NEG[li][:, :], op=ALU.add)
                nc.scalar.dma_start(dram_out[f"vn{lvl}"][:, :], VN[li][:, :])

            nc.scalar.dma_start(dram_out["stats"][:, :], stats[:, :])

    nc.compile()
    return nc


def prep_core_inputs(inputs, core):
    """Shard + relayout one core's inputs."""
    b0 = core * B_PER_CORE
    out = {}
    gtm_parts = []
    cls_parts = []
    for li, (lvl, H, W, nch) in enumerate(LEVELS):
        F = FS[li]
        Fc = F // nch

        def relayout(X, dtype):
            # channel-major: [128, C*F]
            C = X.shape[1]
            Y = X.transpose(1, 0, 2, 3).reshape(C, 128, F)
            return Y.transpose(1, 0, 2).reshape(128, C * F).astype(dtype)

        def relayout_ki(X, dtype, n):
            # channel-innermost per chunk: [n, 128, Fc*C]
            C = X.shape[1]
            Y = X.transpose(1, 0, 2, 3).reshape(C, 128, n, F // n)
            return (Y.transpose(2, 1, 3, 0)
                    .reshape(n, 128, (F // n) * C).astype(dtype))

        cls = inputs[f"cls{lvl}"][b0:b0 + B_PER_CORE]
        gt = inputs[f"gt{lvl}"][b0:b0 + B_PER_CORE]
        reg = inputs[f"reg{lvl}"][b0:b0 + B_PER_CORE]
        gtm_parts.append(relayout(gt[:, 0:3], NP_FP8))
        cls_parts.append(relayout(cls, NP_BF16))
        gx = relayout_ki(gt[:, 3:35], NP_FP8, nch)
        rg = relayout_ki(reg, NP_FP8, nch)
        for j in range(nch):
            out[f"ch{lvl}_{j}"] = np.ascontiguousarray(
                np.concatenate([gx[j], rg[j]], axis=-1))
    out["gtma"] = np.ascontiguousarray(np.concatenate(gtm_parts, axis=1))
    out["clsa"] = np.ascontiguousarray(np.concatenate(cls_parts, axis=1))
    return out


def finish_host(results):
    """Merge per-core device partials into the final [4] loss vector."""
    total = np.zeros(4, dtype=np.float64)
    for li, (lvl, H, W, nch) in enumerate(LEVELS):
        n_pos = neg_cnt = loss_pos = tcl_pos = tcl_all = accx = accy = 0.0
        neg_vals = []
        for r in results:
            st = np.asarray(r["stats"], dtype=np.float64)
            b = 5 * li
            n_pos += st[:, b + C_NPOS].sum()
            neg_cnt += st[:, b + C_NEGCNT].sum()
            loss_pos += st[:, b + C_LOSSPOS].sum()
            tcl_pos += st[:, b + C_TCLPOS].sum()
            tcl_all += st[:, b + C_TCLALL].sum()
            rb = N_LEVEL_COLS + 3 * li
            A = st[:, rb].sum()
            B = st[:, rb + 1].sum()
            D = st[:, rb + 2].sum()
            accx += A - B
            accy += D - A
            v = np.asarray(r[f"vn{lvl}"]).astype(np.float32).ravel()
            neg_vals.append(v[v > 0.0] - 1.0)
        neg_vals = np.concatenate(neg_vals) if neg_vals else np.zeros(0, np.float32)

        M = 16 * H * W
        n_pos_i = int(round(n_pos))
        neg_cnt_i = int(round(neg_cnt))
        if n_pos_i > 0:
            n_neg = min(neg_cnt_i,
                        int(np.floor(np.float32(OHEM_RATIO) * np.float32(n_pos_i))))
        else:
            n_neg = 100
        k = min(n_neg, neg_vals.size)
        if k > 0:
            loss_neg = float(np.partition(neg_vals, neg_vals.size - k)
                             [neg_vals.size - k:].astype(np.float64).sum())
        else:
            loss_neg = 0.0
        loss_tr = (loss_pos + loss_neg) / (n_pos_i + float(n_neg))

        if n_pos_i > 0:
            mean_pos = tcl_pos / max(n_pos_i, 1)
            mean_neg = (tcl_all - tcl_pos) / max(M - n_pos_i, 1)
            loss_tcl = mean_pos + 0.5 * mean_neg
            denom = max(n_pos_i, 1) * KCH
            loss_rx = 0.25 * accx / denom
            loss_ry = 0.25 * accy / denom
        else:
            loss_tcl = loss_rx = loss_ry = 0.0
        total += np.array([loss_tr, loss_tcl, loss_rx, loss_ry])
    return total.astype(np.float32)


_NC_CACHE = None


def _get_nc():
    global _NC_CACHE
    if _NC_CACHE is None:
        _NC_CACHE = build_bass()
    return _NC_CACHE


def run_device(in_maps, trace=False):
    from concourse.bass_utils import run_bass_kernel_spmd
    nc = _get_nc()
    return run_bass_kernel_spmd(nc, in_maps, list(range(NCORES)), trace=trace)


def kernel(**inputs) -> np.ndarray:
    in_maps = [prep_core_inputs(inputs, c) for c in range(NCORES)]
    res = run_device(in_maps)
    return finish_host(res.results)


# revision 10
# speedup vs baseline: 1.1121x; 1.1121x over previous
"""Trainium2 Bass kernel for nn_BSLoss (text-snake style OHEM loss), 8-core
data-parallel.

Strategy
--------
Host shards the batch dim (16 -> 2 per core). gtm masks ship fp8-e3m4 (0/1
exact), cls logits bf16, and the 32 regression channels (map + pred) ship
fp8-e3m4 channel-innermost per pixel, with map and pred concatenated per
chunk ([128, 64*Fc] = [gtx | reg]) so each chunk is ONE fat-row DMA.

Device per core:
  - DVE: masks + counts (in the pre-data idle window), then one fused custom
    op per chunk computing q = 2*smooth_l1(gtx-reg) AND a continuous running
    sum (scan) in a single 1x pass; chunks chain their cumsum via the scan's
    s0 init read from the previous chunk's last column, so each level has one
    continuous cumsum C. Per-level weighted totals come Abel-style from three
    small strided reductions: A = sum_f w_f C[32f+15], B = sum_{f>=1} w_f
    C[32f-1], D = sum_f w_f C[32f+31]; host computes x = A-B, y = D-A.
  - GPSIMD: logit diffs, d_ce, pos*ce products, vn = (ce+1)*neg.
  - ScalarE: sgn, exp, ln(1+x) (2-class CE), and the pos*ce accumulations.
Host merges partials, does the exact global top-k OHEM over the masked
negative CE values (vn, bf16) and the final divisions.
"""

import numpy as np
import ml_dtypes

import concourse.bacc as bacc
import concourse.mybir as mybir
import concourse.dve_ops as dve_ops
from concourse.dve_spec import (
    Spec, Src0, Src1, C0, Zero, One, AluOp, Bin, minn, scan, lower, _has_src1,
)
from concourse.dve_uop import DveOpSpec
from concourse import tile

F32 = mybir.dt.float32
BF16 = mybir.dt.bfloat16
FP8 = mybir.dt.float8e3
NP_BF16 = ml_dtypes.bfloat16
NP_FP8 = ml_dtypes.float8_e3m4
ALU = mybir.AluOpType
ACT = mybir.ActivationFunctionType

NCORES = 8
B_PER_CORE = 2
# level -> (H, W, nchunks)
LEVELS = [(3, 160, 160, 4), (4, 80, 80, 1), (5, 40, 40, 1)]
FS = [B_PER_CORE * H * W // 128 for _, H, W, _ in LEVELS]   # [400, 100, 25]
KCH = 16          # regression channels per axis
OHEM_RATIO = 3.0

# stats tile column layout: per-level block of 5, then 3 cols per level
# (A, B, D weighted-cumsum reductions). Every column is written exactly once.
C_NPOS, C_NEGCNT, C_LOSSPOS, C_TCLPOS, C_TCLALL = range(5)
N_LEVEL_COLS = 5 * len(LEVELS)
STATS_COLS = N_LEVEL_COLS + 3 * len(LEVELS)


def _np_sl1q(d):
    a = np.abs(d)
    m = np.minimum(a, 1.0)
    return m * (a + a - m)   # == 2 * smooth_l1(d)


def _register_custom_ops():
    """Register our fused DVE ops (idempotent)."""
    # QSL1CS: out = s0 + cumsum_freedim(q(Src0 - Src1)),
    #         q(d) = min(|d|,1)*(2|d| - min(|d|,1))
    a = Bin(AluOp.ABSOLUTE_DIFF, Src0, Src1)
    m = minn(a, One)
    q = ((a + a) - m) * m

    def _qs_ref(in0, in1, s0, s1, imm2):
        p = in0.shape[0]
        qq = _np_sl1q(in0.reshape(p, -1).astype(np.float32)
                      - in1.reshape(p, -1).astype(np.float32))
        init = np.asarray(s0).reshape(-1, 1) if isinstance(s0, np.ndarray) else s0
        return init + np.cumsum(qq, axis=1)

    spec_qs = Spec(body=scan(AluOp.ADD, q, init=C0), reference=_qs_ref)

    def _acc_ref(fn):
        def ref(in0, in1, s0, s1, imm2):
            p = in0.shape[0]
            o = fn(in0.reshape(p, -1).astype(np.float32),
                   in1.reshape(p, -1).astype(np.float32) if in1 is not None
                   else None)
            init = np.asarray(s0).reshape(-1, 1) if isinstance(s0, np.ndarray) else s0
            return o, init + o.sum(axis=1, keepdims=True)
        return ref

    # MULR: out = in0*in1 ; accum = s0 + sum(out)
    spec_mulr = Spec(body=Src0 * Src1, accum=AluOp.ADD, accum_init=C0,
                     reference=_acc_ref(lambda a_, b_: a_ * b_))
    # NEGM: out = (1-in0)*in1 ; accum = s0 + sum(out)
    spec_negm = Spec(body=(One - Src0) * Src1, accum=AluOp.ADD, accum_init=C0,
                     reference=_acc_ref(lambda a_, b_: (1.0 - a_) * b_))

    ops = {}
    for name, spec in (("QSL1CS_ANT", spec_qs), ("MULR_ANT", spec_mulr),
                       ("NEGM_ANT", spec_negm)):
        if name in dve_ops._SUB_OPCODE_FOR_NAME:
            ops[name] = next(o for o in dve_ops.OPS if o.name == name)
            continue
        row = dve_ops._CUSTOM_DVE_ROW_BASE + len(dve_ops.OPS)
        shas = {}
        for ver in ("v3", "v4"):
            u = lower(spec, ver=ver)
            shas[ver] = DveOpSpec(name=name, opcode=row, uops=u,
                                  rd1_en=_has_src1(spec)).sha(ver)
        op = dve_ops.DveOp(name, spec, subdim=False, uops_sha=shas)
        dve_ops.OPS.append(op)
        dve_ops.CUSTOM_DVE_SPECS[name] = spec
        dve_ops._SUB_OPCODE_FOR_NAME[name] = row
        ops[name] = op
    return ops


def _install_act_root():
    """Restrict the ACT table universe to the one set holding every function
    we use (exp, ln, identity, copy), so walrus never ping-pongs table sets."""
    import os, json, shutil, tempfile
    if os.environ.get("BASS_ACT_ROOT_JSON_PATH"):
        return
    try:
        from neuronxcc.driver.Job import Job
        from neuronxcc.driver.jobs.support.FindActInfo import findActInfoFile
        src = findActInfoFile(Job.getPackageDir(), "gen3")
        d = json.load(open(src))
        keep = [t for t in d["act_func_sets"]
                if t["name"] == "natural_log_exp_and_others"]
        if not keep:
            return
        tmp = tempfile.mkdtemp(prefix="act_root_")
        srcdir = os.path.dirname(src)
        for t in keep:
            for k in d["pwp_file_keys"]:
                shutil.copy(os.path.join(srcdir, t[k]), tmp)
        with open(os.path.join(tmp, "act_info.json"), "w") as f:
            json.dump({"pwp_file_keys": d["pwp_file_keys"],
                       "act_func_sets": keep}, f)
        os.environ["BASS_ACT_ROOT_JSON_PATH"] = os.path.join(tmp, "act_info.json")
        # Make bass's pre-placed LoadActFuncSet ids consistent with the
        # stripped act_info: patch the table universe to the single set.
        import concourse.hw_specs as hw_specs
        _orig_gat = hw_specs.get_activation_tables

        def _gat(module_arch):
            full = _orig_gat(module_arch)
            return {"natural_log_exp_and_others":
                    full["natural_log_exp_and_others"]}

        hw_specs.get_activation_tables = _gat
        import concourse.bacc as _bacc_mod
        _bacc_mod.get_activation_tables = _gat
        import concourse.bass_interp as _bi_mod
        _bi_mod.get_activation_tables = _gat
    except Exception:
        pass


def build_bass():
    """Build the SPMD Bass module (one core's program)."""
    _install_act_root()
    ops = _register_custom_ops()
    nc = bacc.Bacc("TRN2")

    GTM_COLS = 3 * sum(FS)            # 1575
    CLS_COLS = 4 * sum(FS)            # 2100

    dram_in = {}
    dram_out = {}
    dram_in["gtma"] = nc.dram_tensor("gtma", [128, GTM_COLS], FP8,
                                     kind="ExternalInput")
    dram_in["clsa"] = nc.dram_tensor("clsa", [128, CLS_COLS], BF16,
                                     kind="ExternalInput")
    for lvl, H, W, nch in LEVELS:
        F = B_PER_CORE * H * W // 128
        Fc = F // nch
        for j in range(nch):
            dram_in[f"ch{lvl}_{j}"] = nc.dram_tensor(
                f"ch{lvl}_{j}", [128, 64 * Fc], FP8, kind="ExternalInput")
        dram_out[f"vn{lvl}"] = nc.dram_tensor(
            f"vn{lvl}", [128, F], BF16, kind="ExternalOutput")
    dram_out["stats"] = nc.dram_tensor(
        "stats", [128, STATS_COLS], F32, kind="ExternalOutput")

    QSL1CS, MULR, NEGM = ops["QSL1CS_ANT"], ops["MULR_ANT"], ops["NEGM_ANT"]

    with tile.TileContext(nc) as tc:
        with (
            tc.tile_pool(name="io", bufs=1) as io,
            tc.tile_pool(name="lv", bufs=1) as lv,
            tc.tile_pool(name="wk", bufs=1) as wk,
            tc.tile_pool(name="st", bufs=1) as stp,
        ):
            stats = stp.tile([128, STATS_COLS], F32, name="stats_t")

            # ---- loads: gtm first, then chunks (L3..L5), cls last ----
            GTMA = lv.tile([128, GTM_COLS], FP8, tag="gtma", name="gtma_t")
            CLSA = lv.tile([128, CLS_COLS], BF16, tag="clsa", name="clsa_t")
            CH = {}
            for lvl, H, W, nch in LEVELS:
                F = B_PER_CORE * H * W // 128
                Fc = F // nch
                for j in range(nch):
                    CH[(lvl, j)] = io.tile([128, 64 * Fc], FP8,
                                           tag=f"ch{lvl}_{j}",
                                           name=f"ch_{lvl}_{j}")
            nc.sync.dma_start(GTMA[:, :], dram_in["gtma"][:, :])
            nc.sync.dma_start(CLSA[:, :], dram_in["clsa"][:, :])
            for lvl, H, W, nch in LEVELS:
                for j in range(nch):
                    nc.sync.dma_start(CH[(lvl, j)][:, :],
                                      dram_in[f"ch{lvl}_{j}"][:, :])

            # per-level views into gtma/clsa
            goff = [3 * sum(FS[:i]) for i in range(len(LEVELS))]
            coff = [4 * sum(FS[:i]) for i in range(len(LEVELS))]

            # ---- level tiles ----
            POS, NEG, W2, SGN, DIFF, DCE, EXPD, CE, CESC, VN, QT = (
                {}, {}, {}, {}, {}, {}, {}, {}, {}, {}, {})
            for li, (lvl, H, W, nch) in enumerate(LEVELS):
                F = FS[li]
                POS[li] = lv.tile([128, F], F32, tag=f"pos{li}", name=f"pos_{lvl}")
                NEG[li] = lv.tile([128, F], F32, tag=f"neg{li}", name=f"neg_{lvl}")
                W2[li] = lv.tile([128, F], F32, tag=f"w2{li}", name=f"w2_{lvl}")
                SGN[li] = lv.tile([128, 2 * F], BF16, tag=f"sgn{li}",
                                  name=f"sgn_{lvl}")
                DIFF[li] = lv.tile([128, 2 * F], BF16, tag=f"diff{li}",
                                   name=f"diff_{lvl}")
                DCE[li] = lv.tile([128, 2 * F], BF16, tag=f"dce{li}",
                                  name=f"dce_{lvl}")
                EXPD[li] = lv.tile([128, 2 * F], F32, tag=f"expd{li}",
                                   name=f"expd_{lvl}")
                CE[li] = lv.tile([128, 2 * F], F32, tag=f"ce{li}",
                                 name=f"ce_{lvl}")
                CESC[li] = lv.tile([128, 2 * F], F32, tag=f"cesc{li}",
                                   name=f"cesc_{lvl}")
                VN[li] = lv.tile([128, F], BF16, tag=f"vn{li}", name=f"vn_{lvl}")
                QT[li] = wk.tile([128, 32 * F], F32, tag=f"q{li}",
                                 name=f"q_{lvl}")

            # ---- DVE: masks + counts for all levels (pre-data window) ----
            for li, (lvl, H, W, nch) in enumerate(LEVELS):
                F = FS[li]
                g0 = goff[li]
                tr = GTMA[:, g0:g0 + F]
                train = GTMA[:, g0 + 2 * F:g0 + 3 * F]
                tcl = GTMA[:, g0 + F:g0 + 2 * F]
                base = 5 * li
                nc.vector._custom_dve(
                    MULR, out=POS[li][:, :], in0=tr, in1=train, s0=0.0,
                    accum_out=stats[:, base + C_NPOS:base + C_NPOS + 1])
                nc.vector._custom_dve(
                    NEGM, out=NEG[li][:, :], in0=tr, in1=train, s0=0.0,
                    accum_out=stats[:, base + C_NEGCNT:base + C_NEGCNT + 1])
                # w2 = (1 + tcl) * pos  ==  (tr + tcl) * pos  for 0/1 masks
                nc.vector.scalar_tensor_tensor(
                    out=W2[li][:, :], in0=tcl, scalar=1.0, in1=POS[li][:, :],
                    op0=ALU.add, op1=ALU.mult)

            # ---- ScalarE: sgn for all levels (needs gtm only) ----
            for li, (lvl, H, W, nch) in enumerate(LEVELS):
                F = FS[li]
                g0 = goff[li]
                nc.scalar.activation(SGN[li][:, :], GTMA[:, g0:g0 + 2 * F],
                                     ACT.Identity, bias=1.0, scale=-2.0)

            # ---- DVE diff/dce + ScalarE exp/ln per level ----
            for li, (lvl, H, W, nch) in enumerate(LEVELS):
                F = FS[li]
                c0 = coff[li]
                base = 5 * li
                cls3d = CLSA[:, c0:c0 + 4 * F].rearrange(
                    "p (g f) -> p g f", g=2)
                nc.vector.tensor_tensor(
                    out=DIFF[li][:, :].rearrange("p (g f) -> p g f", g=2),
                    in0=cls3d[:, :, F:2 * F], in1=cls3d[:, :, 0:F],
                    op=ALU.subtract)
                nc.vector.tensor_mul(DCE[li][:, :], DIFF[li][:, :],
                                     SGN[li][:, :])
                nc.scalar.activation(EXPD[li][:, :], DCE[li][:, :], ACT.Exp)
                nc.scalar.activation(CE[li][:, 0:F], EXPD[li][:, 0:F],
                                     ACT.Ln, bias=1.0)
                nc.scalar.activation(
                    CE[li][:, F:2 * F], EXPD[li][:, F:2 * F], ACT.Ln,
                    bias=1.0,
                    accum_out=stats[:, base + C_TCLALL:base + C_TCLALL + 1])

            # ---- DVE: per-chunk fused q+cumsum (chained), ce stats, vn,
            # ---- then per-level trios (2D strided weighted reductions)
            def cesc_vn(li, lvl):
                F = FS[li]
                base = 5 * li
                nc.vector._custom_dve(
                    MULR, out=CESC[li][:, 0:F], in0=POS[li][:, :],
                    in1=CE[li][:, 0:F], s0=0.0,
                    accum_out=stats[:, base + C_LOSSPOS:base + C_LOSSPOS + 1])
                nc.vector._custom_dve(
                    MULR, out=CESC[li][:, F:2 * F], in0=POS[li][:, :],
                    in1=CE[li][:, F:2 * F], s0=0.0,
                    accum_out=stats[:, base + C_TCLPOS:base + C_TCLPOS + 1])
                nc.vector.scalar_tensor_tensor(
                    out=VN[li][:, :], in0=CE[li][:, 0:F], scalar=1.0,
                    in1=NEG[li][:, :], op0=ALU.add, op1=ALU.mult)
                nc.scalar.dma_start(dram_out[f"vn{lvl}"][:, :], VN[li][:, :])

            def trio(li):
                F = FS[li]
                rb = N_LEVEL_COLS + 3 * li
                Q = QT[li]
                scr = wk.tile([128, F], F32, tag=f"scr{li}",
                              name=f"scr_{LEVELS[li][0]}")
                # A = sum_f w_f C[32f+15]
                nc.vector._custom_dve(
                    MULR, out=scr[:, :], in0=Q[:, 15::32],
                    in1=W2[li][:, :], s0=0.0, accum_out=stats[:, rb:rb + 1])
                # B = sum_{f>=1} w_f C[32(f-1)+31]
                nc.vector._custom_dve(
                    MULR, out=scr[:, 0:F - 1], in0=Q[:, 31:32 * (F - 1):32],
                    in1=W2[li][:, 1:F], s0=0.0,
                    accum_out=stats[:, rb + 1:rb + 2])
                # D = sum_f w_f C[32f+31]
                nc.vector._custom_dve(
                    MULR, out=scr[:, :], in0=Q[:, 31::32],
                    in1=W2[li][:, :], s0=0.0,
                    accum_out=stats[:, rb + 2:rb + 3])

            for li, (lvl, H, W, nch) in enumerate(LEVELS):
                F = FS[li]
                Fc = F // nch
                Q = QT[li]
                for j in range(nch):
                    s0 = (0.0 if j == 0
                          else Q[:, j * 32 * Fc - 1:j * 32 * Fc])
                    nc.vector._custom_dve(
                        QSL1CS, out=Q[:, j * 32 * Fc:(j + 1) * 32 * Fc],
                        in0=CH[(lvl, j)][:, 0:32 * Fc],
                        in1=CH[(lvl, j)][:, 32 * Fc:64 * Fc], s0=s0)
                    if li == 0 and j == 0:
                        cesc_vn(0, 3)       # ce_tr/tcl stats + vn for L3
                trio(li)
                if li > 0:
                    cesc_vn(li, lvl)

            nc.scalar.dma_start(dram_out["stats"][:, :], stats[:, :])

    nc.compile()
    return nc


def prep_core_inputs(inputs, core):
    """Shard + relayout one core's inputs."""
    b0 = core * B_PER_CORE
    out = {}
    gtm_parts = []
    cls_parts = []
    for li, (lvl, H, W, nch) in enumerate(LEVELS):
        F = FS[li]
        Fc = F // nch

        def relayout(X, dtype):
            # channel-major: [128, C*F]
            C = X.shape[1]
            Y = X.transpose(1, 0, 2, 3).reshape(C, 128, F)
            return Y.transpose(1, 0, 2).reshape(128, C * F).astype(dtype)

        def relayout_ki(X, dtype, n):
            # channel-innermost per chunk: [n, 128, Fc*C]
            C = X.shape[1]
            Y = X.transpose(1, 0, 2, 3).reshape(C, 128, n, F // n)
            return (Y.transpose(2, 1, 3, 0)
                    .reshape(n, 128, (F // n) * C).astype(dtype))

        cls = inputs[f"cls{lvl}"][b0:b0 + B_PER_CORE]
        gt = inputs[f"gt{lvl}"][b0:b0 + B_PER_CORE]
        reg = inputs[f"reg{lvl}"][b0:b0 + B_PER_CORE]
        gtm_parts.append(relayout(gt[:, 0:3], NP_FP8))
        cls_parts.append(relayout(cls, NP_BF16))
        gx = relayout_ki(gt[:, 3:35], NP_FP8, nch)
        rg = relayout_ki(reg, NP_FP8, nch)
        for j in range(nch):
            out[f"ch{lvl}_{j}"] = np.ascontiguousarray(
                np.concatenate([gx[j], rg[j]], axis=-1))
    out["gtma"] = np.ascontiguousarray(np.concatenate(gtm_parts, axis=1))
    out["clsa"] = np.ascontiguousarray(np.concatenate(cls_parts, axis=1))
    return out


def finish_host(results):
    """Merge per-core device partials into the final [4] loss vector."""
    total = np.zeros(4, dtype=np.float64)
    for li, (lvl, H, W, nch) in enumerate(LEVELS):
        n_pos = neg_cnt = loss_pos = tcl_pos = tcl_all = accx = accy = 0.0
        neg_vals = []
        for r in results:
            st = np.asarray(r["stats"], dtype=np.float64)
            b = 5 * li
            n_pos += st[:, b + C_NPOS].sum()
            neg_cnt += st[:, b + C_NEGCNT].sum()
            loss_pos += st[:, b + C_LOSSPOS].sum()
            tcl_pos += st[:, b + C_TCLPOS].sum()
            tcl_all += st[:, b + C_TCLALL].sum()
            rb = N_LEVEL_COLS + 3 * li
            A = st[:, rb].sum()
            B = st[:, rb + 1].sum()
            D = st[:, rb + 2].sum()
            accx += A - B
            accy += D - A
            v = np.asarray(r[f"vn{lvl}"]).astype(np.float32).ravel()
            neg_vals.append(v[v > 0.0] - 1.0)
        neg_vals = np.concatenate(neg_vals) if neg_vals else np.zeros(0, np.float32)

        M = 16 * H * W
        n_pos_i = int(round(n_pos))
        neg_cnt_i = int(round(neg_cnt))
        if n_pos_i > 0:
            n_neg = min(neg_cnt_i,
                        int(np.floor(np.float32(OHEM_RATIO) * np.float32(n_pos_i))))
        else:
            n_neg = 100
        k = min(n_neg, neg_vals.size)
        if k > 0:
            loss_neg = float(np.partition(neg_vals, neg_vals.size - k)
                             [neg_vals.size - k:].astype(np.float64).sum())
        else:
            loss_neg = 0.0
        loss_tr = (loss_pos + loss_neg) / (n_pos_i + float(n_neg))

        if n_pos_i > 0:
            mean_pos = tcl_pos / max(n_pos_i, 1)
            mean_neg = (tcl_all - tcl_pos) / max(M - n_pos_i, 1)
            loss_tcl = mean_pos + 0.5 * mean_neg
            denom = max(n_pos_i, 1) * KCH
            loss_rx = 0.25 * accx / denom
            loss_ry = 0.25 * accy / denom
        else:
            loss_tcl = loss_rx = loss_ry = 0.0
        total += np.array([loss_tr, loss_tcl, loss_rx, loss_ry])
    return total.astype(np.float32)


_NC_CACHE = None


def _get_nc():
    global _NC_CACHE
    if _NC_CACHE is None:
        _NC_CACHE = build_bass()
    return _NC_CACHE


def run_device(in_maps, trace=False):
    from concourse.bass_utils import run_bass_kernel_spmd
    nc = _get_nc()
    return run_bass_kernel_spmd(nc, in_maps, list(range(NCORES)), trace=trace)


def kernel(**inputs) -> np.ndarray:
    in_maps = [prep_core_inputs(inputs, c) for c in range(NCORES)]
    res = run_device(in_maps)
    return finish_host(res.results)


# revision 12
# speedup vs baseline: 1.1740x; 1.0557x over previous
"""Trainium2 Bass kernel for nn_BSLoss (text-snake style OHEM loss), 8-core
data-parallel.

Strategy
--------
Host shards the batch dim (16 -> 2 per core). Global cross-level layouts:
  - gtm_all [128, 3*FT] bf16 = [tr_all | tcl_all | train_all] (FT=525 is the
    per-partition pixel count over all 3 levels), so masks are ONE fused op
    each over all levels at 2x bf16 throughput.
  - cls_all [128, 4*FT] bf16 = [tr_lo | tcl_lo | tr_hi | tcl_hi], so the CE
    logit diff / dce / exp are single global ops.
  - regression channels (map+pred) ship fp8-e3m4 channel-innermost per pixel,
    [gtx | reg] concatenated per (half-)chunk = one fat-row DMA each, split
    into ~0.4MB pieces so compute starts as soon as the first piece lands.

Device per core:
  - DVE: one fused custom op per chunk-piece computing q = 2*smooth_l1 AND a
    continuous running sum (scan) in one 1x pass; pieces chain their cumsum
    via the scan's s0 init read from the previous piece's last column. The
    per-level weighted regression totals come Abel-style from three strided
    reductions: A = sum_f w_f C[32f+15], B = sum_{f>=1} w_f C[32f-1],
    D = sum_f w_f C[32f+31]; host computes x = A-B, y = D-A.
  - ScalarE: sgn, exp, ln(1+x) (2-class CE), per-level mask counts and the
    tcl-CE accumulations.
Host merges partials, does the exact global top-k OHEM over the masked
negative CE values (vn, bf16) and the final divisions.
"""

import numpy as np
import ml_dtypes

import concourse.bacc as bacc
import concourse.mybir as mybir
import concourse.dve_ops as dve_ops
from concourse.dve_spec import (
    Spec, Src0, Src1, C0, Zero, One, AluOp, Bin, minn, scan, lower, _has_src1,
)
from concourse.dve_uop import DveOpSpec
from concourse import tile

F32 = mybir.dt.float32
BF16 = mybir.dt.bfloat16
FP8 = mybir.dt.float8e3
NP_BF16 = ml_dtypes.bfloat16
NP_FP8 = ml_dtypes.float8_e3m4
ALU = mybir.AluOpType
ACT = mybir.ActivationFunctionType

NCORES = 8
B_PER_CORE = 2
# level -> (H, W, npieces); pieces are the DMA/compute granularity
LEVELS = [(3, 160, 160, 8), (4, 80, 80, 2), (5, 40, 40, 1)]
FS = [B_PER_CORE * H * W // 128 for _, H, W, _ in LEVELS]   # [400, 100, 25]
FT = sum(FS)                                                # 525
LOFF = [sum(FS[:i]) for i in range(len(LEVELS))]            # [0, 400, 500]
KCH = 16
OHEM_RATIO = 3.0

C_NPOS, C_NEGCNT, C_LOSSPOS, C_TCLPOS, C_TCLALL = range(5)
N_LEVEL_COLS = 5 * len(LEVELS)
STATS_COLS = N_LEVEL_COLS + 3 * len(LEVELS)


def _np_sl1q(d):
    a = np.abs(d)
    m = np.minimum(a, 1.0)
    return m * (a + a - m)   # == 2 * smooth_l1(d)


def _register_custom_ops():
    """Register our fused DVE ops (idempotent)."""
    # QSL1CS: out = s0 + cumsum_freedim(q(Src0 - Src1))
    a = Bin(AluOp.ABSOLUTE_DIFF, Src0, Src1)
    m = minn(a, One)
    q = ((a + a) - m) * m

    def _qs_ref(in0, in1, s0, s1, imm2):
        p = in0.shape[0]
        qq = _np_sl1q(in0.reshape(p, -1).astype(np.float32)
                      - in1.reshape(p, -1).astype(np.float32))
        init = np.asarray(s0).reshape(-1, 1) if isinstance(s0, np.ndarray) else s0
        return init + np.cumsum(qq, axis=1)

    spec_qs = Spec(body=scan(AluOp.ADD, q, init=C0), reference=_qs_ref)

    def _acc_ref(fn):
        def ref(in0, in1, s0, s1, imm2):
            p = in0.shape[0]
            o = fn(in0.reshape(p, -1).astype(np.float32),
                   in1.reshape(p, -1).astype(np.float32) if in1 is not None
                   else None)
            init = np.asarray(s0).reshape(-1, 1) if isinstance(s0, np.ndarray) else s0
            return o, init + o.sum(axis=1, keepdims=True)
        return ref

    spec_mulr = Spec(body=Src0 * Src1, accum=AluOp.ADD, accum_init=C0,
                     reference=_acc_ref(lambda a_, b_: a_ * b_))

    ops = {}
    for name, spec in (("QSL1CS_ANT", spec_qs), ("MULR_ANT", spec_mulr)):
        if name in dve_ops._SUB_OPCODE_FOR_NAME:
            ops[name] = next(o for o in dve_ops.OPS if o.name == name)
            continue
        row = dve_ops._CUSTOM_DVE_ROW_BASE + len(dve_ops.OPS)
        shas = {}
        for ver in ("v3", "v4"):
            u = lower(spec, ver=ver)
            shas[ver] = DveOpSpec(name=name, opcode=row, uops=u,
                                  rd1_en=_has_src1(spec)).sha(ver)
        op = dve_ops.DveOp(name, spec, subdim=False, uops_sha=shas)
        dve_ops.OPS.append(op)
        dve_ops.CUSTOM_DVE_SPECS[name] = spec
        dve_ops._SUB_OPCODE_FOR_NAME[name] = row
        ops[name] = op
    return ops


def _install_act_root():
    """Restrict the ACT table universe to the one set holding every function
    we use (exp, ln, identity, copy), so walrus never ping-pongs table sets."""
    import os, json, shutil, tempfile
    if os.environ.get("BASS_ACT_ROOT_JSON_PATH"):
        return
    try:
        from neuronxcc.driver.Job import Job
        from neuronxcc.driver.jobs.support.FindActInfo import findActInfoFile
        src = findActInfoFile(Job.getPackageDir(), "gen3")
        d = json.load(open(src))
        keep = [t for t in d["act_func_sets"]
                if t["name"] == "natural_log_exp_and_others"]
        if not keep:
            return
        tmp = tempfile.mkdtemp(prefix="act_root_")
        srcdir = os.path.dirname(src)
        for t in keep:
            for k in d["pwp_file_keys"]:
                shutil.copy(os.path.join(srcdir, t[k]), tmp)
        with open(os.path.join(tmp, "act_info.json"), "w") as f:
            json.dump({"pwp_file_keys": d["pwp_file_keys"],
                       "act_func_sets": keep}, f)
        os.environ["BASS_ACT_ROOT_JSON_PATH"] = os.path.join(tmp, "act_info.json")
        import concourse.hw_specs as hw_specs
        _orig_gat = hw_specs.get_activation_tables

        def _gat(module_arch):
            full = _orig_gat(module_arch)
            return {"natural_log_exp_and_others":
                    full["natural_log_exp_and_others"]}

        hw_specs.get_activation_tables = _gat
        import concourse.bacc as _bacc_mod
        _bacc_mod.get_activation_tables = _gat
        import concourse.bass_interp as _bi_mod
        _bi_mod.get_activation_tables = _gat
    except Exception:
        pass


# (level, piece) -> piece column count (32*pixels)
def _pieces():
    out = []
    for li, (lvl, H, W, npc) in enumerate(LEVELS):
        F = FS[li]
        assert F % npc == 0
        Fp = F // npc
        for j in range(npc):
            out.append((li, lvl, j, Fp))
    return out


def build_bass():
    """Build the SPMD Bass module (one core's program)."""
    _install_act_root()
    ops = _register_custom_ops()
    nc = bacc.Bacc("TRN2")

    dram_in = {}
    dram_out = {}
    dram_in["gtma"] = nc.dram_tensor("gtma", [128, 3 * FT], BF16,
                                     kind="ExternalInput")
    dram_in["clsa"] = nc.dram_tensor("clsa", [128, 4 * FT], BF16,
                                     kind="ExternalInput")
    for li, lvl, j, Fp in _pieces():
        dram_in[f"ch{lvl}_{j}"] = nc.dram_tensor(
            f"ch{lvl}_{j}", [128, 64 * Fp], FP8, kind="ExternalInput")
    dram_out["vna"] = nc.dram_tensor("vna", [128, FT], BF16,
                                     kind="ExternalOutput")
    dram_out["stats"] = nc.dram_tensor(
        "stats", [128, STATS_COLS], F32, kind="ExternalOutput")

    QSL1CS, MULR = ops["QSL1CS_ANT"], ops["MULR_ANT"]

    with tile.TileContext(nc) as tc:
        with (
            tc.tile_pool(name="io", bufs=1) as io,
            tc.tile_pool(name="lv", bufs=1) as lv,
            tc.tile_pool(name="wk", bufs=1) as wk,
            tc.tile_pool(name="st", bufs=1) as stp,
        ):
            stats = stp.tile([128, STATS_COLS], F32, name="stats_t")

            GTMA = lv.tile([128, 3 * FT], BF16, tag="gtma", name="gtma_t")
            CLSA = lv.tile([128, 4 * FT], BF16, tag="clsa", name="clsa_t")
            CH = {}
            for li, lvl, j, Fp in _pieces():
                CH[(li, j)] = io.tile([128, 64 * Fp], FP8,
                                      tag=f"ch{lvl}_{j}", name=f"ch_{lvl}_{j}")

            # ---- load order: h0 h1 h2 gtm h3 h4 cls h5 h6 h7 c4a c4b c5
            pcs = [(0, 0), (0, 1), (0, 2), "gtm", (0, 3), (0, 4), "cls",
                   (0, 5), (0, 6), (0, 7), (1, 0), (1, 1), (2, 0)]
            for p in pcs:
                if p == "gtm":
                    nc.sync.dma_start(GTMA[:, :], dram_in["gtma"][:, :])
                elif p == "cls":
                    nc.sync.dma_start(CLSA[:, :], dram_in["clsa"][:, :])
                else:
                    li, j = p
                    lvl = LEVELS[li][0]
                    nc.sync.dma_start(CH[(li, j)][:, :],
                                      dram_in[f"ch{lvl}_{j}"][:, :])

            tr_a = GTMA[:, 0:FT]
            tcl_a = GTMA[:, FT:2 * FT]
            train_a = GTMA[:, 2 * FT:3 * FT]

            pos = lv.tile([128, FT], BF16, tag="pos", name="pos_t")
            neg = lv.tile([128, FT], BF16, tag="neg", name="neg_t")
            w2 = lv.tile([128, FT], BF16, tag="w2", name="w2_t")
            sgn = lv.tile([128, 2 * FT], BF16, tag="sgn", name="sgn_t")
            diff = lv.tile([128, 2 * FT], BF16, tag="diff", name="diff_t")
            dce = lv.tile([128, 2 * FT], BF16, tag="dce", name="dce_t")
            expd = lv.tile([128, 2 * FT], F32, tag="expd", name="expd_t")
            ce = lv.tile([128, 2 * FT], BF16, tag="ce", name="ce_t")
            cesc = lv.tile([128, 2 * FT], BF16, tag="cesc", name="cesc_t")
            vna = lv.tile([128, FT], BF16, tag="vna", name="vna_t")
            junk = lv.tile([128, FT], BF16, tag="junk", name="junk_t")
            QT = [wk.tile([128, 32 * FS[li]], F32, tag=f"q{li}",
                          name=f"q_{LEVELS[li][0]}") for li in range(3)]
            SCR = [wk.tile([128, FS[li]], F32, tag=f"scr{li}",
                           name=f"scr_{LEVELS[li][0]}") for li in range(3)]

            # ---- DVE: fused q+cumsum per piece (chained within level) ----
            def qpiece(li, j, Fp):
                Q = QT[li]
                o0 = j * 32 * Fp
                s0 = 0.0 if j == 0 else Q[:, o0 - 1:o0]
                nc.vector._custom_dve(
                    QSL1CS, out=Q[:, o0:o0 + 32 * Fp],
                    in0=CH[(li, j)][:, 0:32 * Fp],
                    in1=CH[(li, j)][:, 32 * Fp:64 * Fp], s0=s0)

            # masks: one fused op each over all levels (bf16, 2x)
            def masks():
                nc.vector.tensor_mul(pos[:, :], tr_a, train_a)
                nc.vector.tensor_tensor(out=neg[:, :], in0=train_a,
                                        in1=pos[:, :], op=ALU.subtract)
                nc.vector.scalar_tensor_tensor(
                    out=w2[:, :], in0=tcl_a, scalar=1.0, in1=pos[:, :],
                    op0=ALU.add, op1=ALU.mult)

            def dd():
                nc.vector.tensor_tensor(
                    out=diff[:, :], in0=CLSA[:, 2 * FT:4 * FT],
                    in1=CLSA[:, 0:2 * FT], op=ALU.subtract)
                nc.vector.tensor_mul(dce[:, :], diff[:, :], sgn[:, :])

            def cesc_ops(li):
                F = FS[li]
                o = LOFF[li]
                base = 5 * li
                nc.vector._custom_dve(
                    MULR, out=cesc[:, o:o + F], in0=pos[:, o:o + F],
                    in1=ce[:, o:o + F], s0=0.0,
                    accum_out=stats[:, base + C_LOSSPOS:base + C_LOSSPOS + 1])
                nc.vector._custom_dve(
                    MULR, out=cesc[:, FT + o:FT + o + F],
                    in0=pos[:, o:o + F], in1=ce[:, FT + o:FT + o + F], s0=0.0,
                    accum_out=stats[:, base + C_TCLPOS:base + C_TCLPOS + 1])

            def vn_op():
                nc.vector.scalar_tensor_tensor(
                    out=vna[:, :], in0=ce[:, 0:FT], scalar=1.0,
                    in1=neg[:, :], op0=ALU.add, op1=ALU.mult)
                nc.scalar.dma_start(dram_out["vna"][:, :], vna[:, :])

            def trio(li):
                F = FS[li]
                o = LOFF[li]
                rb = N_LEVEL_COLS + 3 * li
                Q = QT[li]
                scr = SCR[li]
                nc.vector._custom_dve(
                    MULR, out=scr[:, :], in0=Q[:, 15::32],
                    in1=w2[:, o:o + F], s0=0.0, accum_out=stats[:, rb:rb + 1])
                nc.vector._custom_dve(
                    MULR, out=scr[:, 0:F - 1], in0=w2[:, o + 1:o + F],
                    in1=Q[:, 31:32 * (F - 1):32], s0=0.0,
                    accum_out=stats[:, rb + 1:rb + 2])
                nc.vector._custom_dve(
                    MULR, out=scr[:, :], in0=Q[:, 31::32],
                    in1=w2[:, o:o + F], s0=0.0,
                    accum_out=stats[:, rb + 2:rb + 3])

            # Emission order = Tile's dependency order; interleave engines
            # following the dataflow (scalar writers before DVE readers).
            qpiece(0, 0, 50)
            qpiece(0, 1, 50)
            qpiece(0, 2, 50)
            masks()
            # ScalarE: sgn (needs gtm), per-level counts (need pos/neg)
            nc.scalar.activation(sgn[:, :], GTMA[:, 0:2 * FT],
                                 ACT.Identity, bias=1.0, scale=-2.0)
            for li in range(3):
                F = FS[li]
                o = LOFF[li]
                base = 5 * li
                nc.scalar.activation(
                    junk[:, o:o + F], pos[:, o:o + F], ACT.Identity,
                    accum_out=stats[:, base + C_NPOS:base + C_NPOS + 1])
                nc.scalar.activation(
                    junk[:, o:o + F], neg[:, o:o + F], ACT.Identity,
                    accum_out=stats[:, base + C_NEGCNT:base + C_NEGCNT + 1])
            qpiece(0, 3, 50)
            qpiece(0, 4, 50)
            dd()
            # ScalarE: exp + ln (produce ce)
            nc.scalar.activation(expd[:, :], dce[:, :], ACT.Exp)
            nc.scalar.activation(ce[:, 0:FT], expd[:, 0:FT], ACT.Ln, bias=1.0)
            for li in range(3):
                F = FS[li]
                o = LOFF[li]
                base = 5 * li
                nc.scalar.activation(
                    ce[:, FT + o:FT + o + F], expd[:, FT + o:FT + o + F],
                    ACT.Ln, bias=1.0,
                    accum_out=stats[:, base + C_TCLALL:base + C_TCLALL + 1])
            qpiece(0, 5, 50)
            qpiece(0, 6, 50)
            qpiece(0, 7, 50)
            cesc_ops(0)
            vn_op()
            qpiece(1, 0, 50)
            qpiece(1, 1, 50)
            trio(0)
            qpiece(2, 0, 25)
            cesc_ops(1)
            cesc_ops(2)
            trio(1)
            trio(2)

            nc.scalar.dma_start(dram_out["stats"][:, :], stats[:, :])

    nc.compile()
    return nc


def prep_core_inputs(inputs, core):
    """Shard + relayout one core's inputs."""
    b0 = core * B_PER_CORE
    out = {}
    gtm_ch = [[], [], []]          # tr, tcl, train blocks per level
    cls_ch = [[], [], [], []]      # tr_lo, tcl_lo, tr_hi, tcl_hi
    CLS_ORDER = [0, 2, 1, 3]       # channel idx for (tr_lo, tcl_lo, tr_hi, tcl_hi)
    for li, (lvl, H, W, npc) in enumerate(LEVELS):
        F = FS[li]
        Fp = F // npc

        def chan(X, c):
            # one channel -> [128, F]
            return (X[:, c].reshape(B_PER_CORE * H * W)
                    .reshape(128, F))

        cls = np.asarray(inputs[f"cls{lvl}"][b0:b0 + B_PER_CORE])
        gt = np.asarray(inputs[f"gt{lvl}"][b0:b0 + B_PER_CORE])
        reg = np.asarray(inputs[f"reg{lvl}"][b0:b0 + B_PER_CORE])
        # NOTE: pixel flat order must match chan():
        # X[:, c] is [B, H, W] -> reshape(B*H*W) -> [128, F] row-major.
        for ci in range(3):
            gtm_ch[ci].append(chan(gt, ci))
        for k, ci in enumerate(CLS_ORDER):
            cls_ch[k].append(chan(cls, ci))

        # regression: [n, 128, Fp*32] channel-innermost, gtx|reg concat
        def ki(X):
            C = X.shape[1]          # 32
            Y = X.transpose(1, 0, 2, 3).reshape(C, 128, npc, Fp)
            return Y.transpose(2, 1, 3, 0).reshape(npc, 128, Fp * C)

        gx = ki(gt[:, 3:35])
        rg = ki(reg)
        for j in range(npc):
            out[f"ch{lvl}_{j}"] = np.ascontiguousarray(np.concatenate(
                [gx[j], rg[j]], axis=-1).astype(NP_FP8))

    out["gtma"] = np.ascontiguousarray(np.concatenate(
        [np.concatenate(blocks, axis=1) for blocks in gtm_ch],
        axis=1).astype(NP_BF16))
    out["clsa"] = np.ascontiguousarray(np.concatenate(
        [np.concatenate(blocks, axis=1) for blocks in cls_ch],
        axis=1).astype(NP_BF16))
    return out


def finish_host(results):
    """Merge per-core device partials into the final [4] loss vector."""
    total = np.zeros(4, dtype=np.float64)
    for li, (lvl, H, W, npc) in enumerate(LEVELS):
        F = FS[li]
        o = LOFF[li]
        n_pos = neg_cnt = loss_pos = tcl_pos = tcl_all = accx = accy = 0.0
        neg_vals = []
        for r in results:
            st = np.asarray(r["stats"], dtype=np.float64)
            b = 5 * li
            n_pos += st[:, b + C_NPOS].sum()
            neg_cnt += st[:, b + C_NEGCNT].sum()
            loss_pos += st[:, b + C_LOSSPOS].sum()
            tcl_pos += st[:, b + C_TCLPOS].sum()
            tcl_all += st[:, b + C_TCLALL].sum()
            rb = N_LEVEL_COLS + 3 * li
            A = st[:, rb].sum()
            B = st[:, rb + 1].sum()
            D = st[:, rb + 2].sum()
            accx += A - B
            accy += D - A
            v = (np.asarray(r["vna"])[:, o:o + F]
                 .astype(np.float32).ravel())
            neg_vals.append(v[v > 0.0] - 1.0)
        neg_vals = np.concatenate(neg_vals) if neg_vals else np.zeros(0, np.float32)

        M = 16 * H * W
        n_pos_i = int(round(n_pos))
        neg_cnt_i = int(round(neg_cnt))
        if n_pos_i > 0:
            n_neg = min(neg_cnt_i,
                        int(np.floor(np.float32(OHEM_RATIO) * np.float32(n_pos_i))))
        else:
            n_neg = 100
        k = min(n_neg, neg_vals.size)
        if k > 0:
            loss_neg = float(np.partition(neg_vals, neg_vals.size - k)
                             [neg_vals.size - k:].astype(np.float64).sum())
        else:
            loss_neg = 0.0
        loss_tr = (loss_pos + loss_neg) / (n_pos_i + float(n_neg))

        if n_pos_i > 0:
            mean_pos = tcl_pos / max(n_pos_i, 1)
            mean_neg = (tcl_all - tcl_pos) / max(M - n_pos_i, 1)
            loss_tcl = mean_pos + 0.5 * mean_neg
            denom = max(n_pos_i, 1) * KCH
            loss_rx = 0.25 * accx / denom
            loss_ry = 0.25 * accy / denom
        else:
            loss_tcl = loss_rx = loss_ry = 0.0
        total += np.array([loss_tr, loss_tcl, loss_rx, loss_ry])
    return total.astype(np.float32)


_NC_CACHE = None


def _get_nc():
    global _NC_CACHE
    if _NC_CACHE is None:
        _NC_CACHE = build_bass()
    return _NC_CACHE


def run_device(in_maps, trace=False):
    from concourse.bass_utils import run_bass_kernel_spmd
    nc = _get_nc()
    return run_bass_kernel_spmd(nc, in_maps, list(range(NCORES)), trace=trace)


def kernel(**inputs) -> np.ndarray:
    in_maps = [prep_core_inputs(inputs, c) for c in range(NCORES)]
    res = run_device(in_maps)
    return finish_host(res.results)
